# revision 41
# baseline (speedup 1.0000x reference)
"""MoE layer (routed top-2 experts + shared SwiGLU expert) on 8 TRN2 NeuronCores.

Sharding strategy (per spec hint):
  - Routed experts: expert-parallel. Core e holds W1/W2/W3[e]; the host computes
    the router (bit-matching the reference's jax fp32 computation on CPU), gathers
    each expert's assigned tokens (top-2 of 8 per token => ~T/4 tokens per expert),
    and ships a [C, D] token block per core (C = max expert count). This is exact
    vs. the dense reference since w_full is zero for non-selected experts.
  - Shared expert: data-parallel on tokens. Core e processes tokens
    [e*T/8, (e+1)*T/8) through the full shared SwiGLU (weights replicated).
  - Combine: host scatter-add of weighted routed outputs + shared outputs.

Device kernel per core: two SwiGLU FFN passes (routed block, shared block):
    hT = (W1^T x^T) [H, Ttok]  (PSUM f32, accumulated over D/128 chunks)
    h2T = hT * silu(h3T)       (ACT sigmoid + DVE muls, cast to bf16)
    yT = W2^T h2T              [D, Ttok]  (W2 stationary, tokens streamed)
All matmuls in bf16 with fp32 PSUM accumulation; outputs shipped bf16.

Phase B keeps W2 stationary and streams tokens so the partial (C % 512) token
chunk costs cycles proportional to its width (the token-stationary form streams
all 512 W2 columns per token tile), and so the kernel's final output tile is a
tiny [128, C%512] bf16 block - the post-compute DMA drain is ~1us instead of
~8us for a [67, 512] f32 row-strided block.
"""

from contextlib import ExitStack

import numpy as np
import ml_dtypes

import concourse.bacc as bacc
import concourse.tile as tile
from concourse import mybir
from concourse.bass_utils import run_bass_kernel_spmd

# Problem constants (hardcoded per the self-contained-kernel contract)
B, S, D, H, E, TOPK = 2, 2048, 1024, 2048, 8, 2
SCALE = 1.0 / float(np.sqrt(D))
NCORES = 8
P = 128
BF16 = ml_dtypes.bfloat16

# test.py introspection: last BassKernelResults (exec_time_ns when BASS_TRACE=1)
LAST_RESULTS = None

_NC_CACHE = {}

# Sigmoid+DVE-mul beats the ACT Silu table by ~54us on HW (cold-table cost),
# and CoreSim has no Silu - so the split path is the default everywhere.
SIM_COMPAT_SILU = True

# PE p-state warm-up matmuls emitted before the first real chain. ~85ns each
# while ramping; sized so the warm-up ends about when the lead w1 block lands
# (~14us, descriptor-generation-limited) - idling the PE between the first
# chains both wastes time and makes HAM downshift the clock.
WARMUP_MM = 72


def _ensure_ntff_hook():
    """run_bass_kernel_spmd(trace=True) imports antenv.axon_hooks, which this
    image's antenv lacks. Install a stub (wired to the ctypes NTFF profiler if
    available) so a BASS_TRACE=1 environment doesn't crash the kernel."""
    import sys
    import types

    try:
        import antenv.axon_hooks  # noqa: F401

        return
    except ImportError:
        pass
    try:
        import antenv
    except ImportError:
        return
    mod = types.ModuleType("antenv.axon_hooks")
    holder = [None]
    mod.set_axon_ntff_profile_hook = lambda h: holder.__setitem__(0, h)
    mod.get_axon_ntff_profile_hook = lambda: holder[0]
    sys.modules["antenv.axon_hooks"] = mod
    antenv.axon_hooks = mod
    try:
        import trn_agent_boot.trn_boot as tb

        mod.set_axon_ntff_profile_hook(
            tb._ntff_profile_via_ctypes("/opt/axon/libaxon_pjrt.so")
        )
    except Exception:
        pass
    # In hook-less images the artifact share upload is likely unavailable too;
    # make the trace path's upload best-effort instead of fatal.
    try:
        import concourse.bass_utils as bu

        _orig_upload = bu.upload_artifacts

        def _safe_upload(tmpdir):
            try:
                return _orig_upload(tmpdir)
            except Exception:
                return tmpdir

        bu.upload_artifacts = _safe_upload
    except Exception:
        pass


_ensure_ntff_hook()


def _token_chunks(t, step=512):
    """[(offset, size), ...] covering range(t) in chunks of <=step."""
    out = []
    o = 0
    while o < t:
        out.append((o, min(step, t - o)))
        o += step
    return out


def _emit_ffn(tc, pools, dram, Ttok, out_qs, lead_on_scalar=False):
    """Emit one SwiGLU FFN pass: yT[D,Ttok] = W2^T (x@W1 * silu(x@W3))^T.

    dram: dict with xt [D,Ttok] bf16, w1/w3 [D,H] bf16, w2 [H,D] bf16,
          yt [D,Ttok] bf16 DRAM APs.
    out_qs: engines whose DGE queues carry the output DMAs (rotated).
    lead_on_scalar: first pass only - later passes' leads would queue behind
          the previous pass's output DMAs (gated on mid-kernel evicts).
    """
    nc = tc.nc
    KD = D // P    # contraction chunks for phase A (8)
    MH = H // P    # h tiles (16)
    KH = H // P    # contraction chunks for phase B (16)
    MD = D // P    # output D tiles for phase B (8)
    # 512-wide token chunks everywhere: one PSUM bank per tile. (A 1024-wide
    # f32 PSUM matmul passes the bass-level compile but neuronxcc rejects it.)
    chunksA = _token_chunks(Ttok, 512)
    chunksB = _token_chunks(Ttok, 512)

    xt_d = dram["xt"].rearrange("(k p) t -> k p t", p=P)     # [KD, 128, Ttok]
    w1_d = dram["w1"].rearrange("(k p) h -> k p h", p=P)     # [KD, 128, H]
    w3_d = dram["w3"].rearrange("(k p) h -> k p h", p=P)
    w2_d = dram["w2"].rearrange("(k p) d -> k p d", p=P)     # [KH, 128, D]
    yt_d = dram["yt"]                                        # [D, nfull*512]
    ytail_d = dram.get("ytail")                              # [P, MD*tail] or None

    # Resident SBUF tensors (bufs=1 pools; pass 2 reuses the same slots)
    x_sb = pools["x"].tile([P, KD, Ttok], mybir.dt.bfloat16, tag="x_sb")
    w1_sb = pools["wA"].tile([P, KD, H], mybir.dt.bfloat16, tag="w1_sb")
    w3_sb = pools["wA"].tile([P, KD, H], mybir.dt.bfloat16, tag="w3_sb")
    w2_sb = pools["wB"].tile([P, KH, D], mybir.dt.bfloat16, tag="w2_sb")
    h2t_sb = pools["h2t"].tile([P, KH, Ttok], mybir.dt.bfloat16, tag="h2t_sb")

    # DMA delivery matches phase A's consumption order: x first, then w3/w1 in
    # alternating 512-column blocks (ascending columns, all k per block) -
    # phase A's mi-th tile needs columns [mi*128,(mi+1)*128) of EVERY k, so
    # column-blocked interleave keeps the PE fed from mi=0 on instead of
    # gating mi>=4 on the whole w3 bulk. A starved PE also makes HAM downshift
    # the clock, so smooth delivery is worth ~2x here. Descriptor generation
    # costs ~0.6us per 128-line transfer serialized per sequencer (fusing k's
    # doesn't help - gen scales with lines), and the cold-start lead is
    # gen-limited: for the first pass, w3's lead block plus half of w1's ride
    # the Scalar queue in parallel with x plus the other w1 half on Sync,
    # which lands x ~12us, w3 lead ~13us, w1 lead ~15us - just in time for
    # the first ps3/ps1 chains.
    WB = 512  # weight column block (4 mi tiles)
    for k in range(KD):
        nc.sync.dma_start(out=x_sb[:, k, :], in_=xt_d[k])  # full row: widest lines
    for b0 in range(0, H, WB):
        bsl = slice(b0, b0 + WB)
        lead = b0 == 0 and lead_on_scalar
        for k in range(KD):
            q = nc.scalar if lead else nc.sync
            q.dma_start(out=w3_sb[:, k, bsl], in_=w3_d[k, :, bsl])
        for k in range(KD):
            q = (nc.sync if k < KD // 2 else nc.scalar) if lead else nc.sync
            q.dma_start(out=w1_sb[:, k, bsl], in_=w1_d[k, :, bsl])
    for k in range(KH):
        nc.sync.dma_start(out=w2_sb[:, k, :], in_=w2_d[k])

    # Phase A: h2T[H, Ttok] = (W1^T x^T) * silu(W3^T x^T), bf16.
    # k-outer per h-tile: each stationary weight chunk streams all token chunks
    # (fewer LDWEIGHTS, better hiding). All token-chunk PSUM tiles stay live.
    for mi in range(MH):
        hsl = slice(mi * P, (mi + 1) * P)
        for (o, nw) in chunksA:
            # ps3 accumulates FIRST: its sigmoid+mul evict then overlaps ps1's
            # matmuls, leaving only the final h2t mul exposed after ps1 stops.
            ps3 = pools["psA"].tile([P, 512], mybir.dt.float32, tag="ps3", bufs=2)
            ps1 = pools["psA"].tile([P, 512], mybir.dt.float32, tag="ps1")
            for k in range(KD):
                nc.tensor.matmul(
                    ps3[:, :nw],
                    lhsT=w3_sb[:, k : k + 1, hsl],
                    rhs=x_sb[:, k : k + 1, o : o + nw],
                    start=(k == 0),
                    stop=(k == KD - 1),
                )
            for k in range(KD):
                nc.tensor.matmul(
                    ps1[:, :nw],
                    lhsT=w1_sb[:, k : k + 1, hsl],
                    rhs=x_sb[:, k : k + 1, o : o + nw],
                    start=(k == 0),
                    stop=(k == KD - 1),
                )
            # silu = h3 * sigmoid(h3). The split sigmoid+mul path is both
            # CoreSim-compatible and faster on HW than ACT's Silu table.
            sil = pools["tmp"].tile([P, 512], mybir.dt.float32, tag="sil")
            if SIM_COMPAT_SILU:
                sig = pools["tmp"].tile([P, 512], mybir.dt.float32, tag="sig")
                nc.scalar.activation(
                    sig[:, :nw], ps3[:, :nw], mybir.ActivationFunctionType.Sigmoid
                )
                nc.vector.tensor_mul(sil[:, :nw], ps3[:, :nw], sig[:, :nw])
            else:
                nc.scalar.activation(
                    sil[:, :nw], ps3[:, :nw], mybir.ActivationFunctionType.Silu
                )
            nc.vector.tensor_mul(h2t_sb[:, mi, o : o + nw], ps1[:, :nw], sil[:, :nw])

    # Phase B: yT[D, Ttok] = W2^T h2T. W2 stationary, tokens streamed, so the
    # partial last chunk costs ~nw/512 of a full tile and ends the pass with a
    # small output DMA. chunk-outer / d-tile-inner; 16-matmul PSUM chains.
    qi = 0
    for (o, nw) in chunksB:
        partial = nw < 512
        if partial:
            # Partial-chunk outputs pack into one SBUF tile and ship as a
            # single [128, MD*nw] DMA: per-dt [128, nw] stores would emit
            # 128 descriptors of nw*2-byte lines each - a ~4us post-compute
            # drain for the kernel's very last tile.
            ypack = pools["tmp"].tile([P, MD * nw], mybir.dt.bfloat16, tag="ypack")
        for dt in range(MD):
            dsl = slice(dt * P, (dt + 1) * P)
            ps = pools["psB"].tile([P, 512], mybir.dt.float32, tag="psB")
            for k in range(KH):
                nc.tensor.matmul(
                    ps[:, :nw],
                    lhsT=w2_sb[:, k : k + 1, dsl],
                    rhs=h2t_sb[:, k : k + 1, o : o + nw],
                    start=(k == 0),
                    stop=(k == KH - 1),
                )
            # Output DMAs ride the GpSimd ring exclusively: descriptor gen on
            # the Scalar queue would block ACT's sigmoid stream (same
            # sequencer) right at the B->A pass transition, and Sync carries
            # the input weight stream.
            if partial:
                nc.vector.tensor_copy(
                    out=ypack[:, dt * nw : (dt + 1) * nw], in_=ps[:, :nw]
                )
            else:
                yt = pools["tmp"].tile([P, 512], mybir.dt.bfloat16, tag="yt")
                nc.vector.tensor_copy(out=yt[:, :nw], in_=ps[:, :nw])
                q = out_qs[qi % len(out_qs)]
                qi += 1
                q.dma_start(out=yt_d[dsl, o : o + nw], in_=yt[:, :nw])
        if partial:
            # Two halves so the first DMA's descriptor gen overlaps the
            # remaining pack casts instead of waiting for all MD of them.
            half = (MD // 2) * nw
            out_qs[0].dma_start(out=ytail_d[:, :half], in_=ypack[:, :half])
            out_qs[-1].dma_start(out=ytail_d[:, half:], in_=ypack[:, half:])


def _build_nc(C, SS):
    """Build the per-core Bass program: shared FFN ([SS] tokens) + routed FFN ([C])."""
    nc = bacc.Bacc("TRN2", target_bir_lowering=False, debug=False)

    bf = mybir.dt.bfloat16
    MD = D // P
    CF, CT = (C // 512) * 512, C % 512    # full-chunk cols, tail cols
    routed = {
        "xt": nc.dram_tensor("xgt", [D, C], bf, kind="ExternalInput").ap(),
        "w1": nc.dram_tensor("w1", [D, H], bf, kind="ExternalInput").ap(),
        "w3": nc.dram_tensor("w3", [D, H], bf, kind="ExternalInput").ap(),
        "w2": nc.dram_tensor("w2", [H, D], bf, kind="ExternalInput").ap(),
        "yt": nc.dram_tensor("yg", [D, max(CF, 1)], bf, kind="ExternalOutput").ap(),
    }
    if CT:
        routed["ytail"] = nc.dram_tensor(
            "ygtail", [P, MD * CT], bf, kind="ExternalOutput"
        ).ap()
    assert SS % 512 == 0, "shared block must be whole 512-token chunks"
    shared = {
        "xt": nc.dram_tensor("xst", [D, SS], bf, kind="ExternalInput").ap(),
        "w1": nc.dram_tensor("ws1", [D, H], bf, kind="ExternalInput").ap(),
        "w3": nc.dram_tensor("ws3", [D, H], bf, kind="ExternalInput").ap(),
        "w2": nc.dram_tensor("ws2", [H, D], bf, kind="ExternalInput").ap(),
        "yt": nc.dram_tensor("ys", [D, SS], bf, kind="ExternalOutput").ap(),
    }

    with tile.TileContext(nc) as tc, ExitStack() as ctx:
        pools = {
            "x": ctx.enter_context(tc.tile_pool(name="x", bufs=1)),
            "wA": ctx.enter_context(tc.tile_pool(name="wA", bufs=1)),
            "wB": ctx.enter_context(tc.tile_pool(name="wB", bufs=1)),
            "h2t": ctx.enter_context(tc.tile_pool(name="h2t", bufs=1)),
            "tmp": ctx.enter_context(tc.tile_pool(name="tmp", bufs=4)),
            "psA": ctx.enter_context(tc.tile_pool(name="psA", bufs=3, space="PSUM")),
            "psB": ctx.enter_context(tc.tile_pool(name="psB", bufs=3, space="PSUM")),
        }
        # HAM warm-up: dummy matmuls on a zeroed tile while the input DMAs
        # stream in, so the PE clock-gate is ramped when real work starts.
        warm = pools["tmp"].tile([P, P], mybir.dt.bfloat16, tag="warm")
        nc.vector.memset(warm[:], 0.0)
        wps = pools["psA"].tile([P, P], mybir.dt.float32, tag="ps1", name="wps")
        for i in range(WARMUP_MM):
            nc.tensor.matmul(wps[:], lhsT=warm[:], rhs=warm[:], start=True, stop=True)
        # Shared-pass outputs stay off the Scalar ring (its descriptor gen
        # blocks ACT's sigmoids, which the routed A phase still needs); by
        # the routed B phase ACT is idle, so rotating two rings there halves
        # the post-compute output drain.
        _emit_ffn(tc, pools, shared, SS, [nc.gpsimd], lead_on_scalar=True)
        _emit_ffn(tc, pools, routed, C, [nc.scalar, nc.gpsimd])

    nc.compile()
    return nc


def _route(x, Wr, rb):
    """Replicate the reference router. Returns (idx [T,2] int, w [T,2] f32).

    Uses jax on CPU with the exact expressions from the reference so the top-2
    selection bit-matches a CPU-run reference (min 2nd-vs-3rd logit gap in this
    problem is ~1e-6, so the selection must match the reference's fp32 math).
    Falls back to numpy float64 if jax-cpu is unavailable.
    """
    try:
        import jax
        import jax.numpy as jnp

        cpu = jax.devices("cpu")[0]
        with jax.default_device(cpu):
            xl = jnp.asarray(np.asarray(x))
            wr = jnp.asarray(np.asarray(Wr))
            rbj = jnp.asarray(np.asarray(rb))
            logits = jnp.einsum("bsd,de->bse", xl, wr) * SCALE
            _, idx = jax.lax.top_k(logits + rbj, TOPK)
            gathered = jnp.take_along_axis(logits, idx, axis=-1)
            w = jax.nn.softmax(gathered, axis=-1)
        idx = np.asarray(idx).reshape(-1, TOPK)
        w = np.asarray(w, dtype=np.float32).reshape(-1, TOPK)
        return idx, w
    except Exception:
        xf = np.asarray(x, np.float64).reshape(-1, D)
        logits = (xf @ np.asarray(Wr, np.float64)) * SCALE
        biased = logits + np.asarray(rb, np.float64)
        idx = np.argsort(-biased, axis=-1)[:, :TOPK]
        g = np.take_along_axis(logits, idx, axis=-1)
        g = g - g.max(axis=-1, keepdims=True)
        wexp = np.exp(g)
        w = (wexp / wexp.sum(axis=-1, keepdims=True)).astype(np.float32)
        return idx, w


def kernel(x, Wr, rb, W1, W2, W3, Ws1, Ws2, Ws3):
    global LAST_RESULTS
    x = np.asarray(x, np.float32)
    Wr = np.asarray(Wr, np.float32)
    rb = np.asarray(rb, np.float32)
    W1 = np.asarray(W1, np.float32)
    W2 = np.asarray(W2, np.float32)
    W3 = np.asarray(W3, np.float32)
    Ws1 = np.asarray(Ws1, np.float32)
    Ws2 = np.asarray(Ws2, np.float32)
    Ws3 = np.asarray(Ws3, np.float32)

    T = B * S
    xf = x.reshape(T, D)

    # ---- Router (host, exact) ----
    idx, w = _route(x, Wr, rb)

    # ---- Shard ----
    toks = [np.nonzero((idx == e).any(axis=1))[0] for e in range(E)]
    wtok = [
        w[toks[e], :][idx[toks[e], :] == e].astype(np.float32) for e in range(E)
    ]
    counts = [len(t) for t in toks]
    C = max(256, max(counts))  # exact max count; matmul free dims need no alignment
    SS = T // NCORES

    xf_bf = xf.astype(BF16)
    in_maps = []
    for e in range(E):
        xg = np.zeros((C, D), dtype=BF16)
        xg[: counts[e]] = xf_bf[toks[e]]
        in_maps.append(
            {
                "xgt": np.ascontiguousarray(xg.T),
                "w1": np.ascontiguousarray(W1[e].astype(BF16)),
                "w3": np.ascontiguousarray(W3[e].astype(BF16)),
                "w2": np.ascontiguousarray(W2[e].astype(BF16)),
                "xst": np.ascontiguousarray(xf_bf[e * SS : (e + 1) * SS].T),
                "ws1": np.ascontiguousarray(Ws1.astype(BF16)),
                "ws3": np.ascontiguousarray(Ws3.astype(BF16)),
                "ws2": np.ascontiguousarray(Ws2.astype(BF16)),
            }
        )

    # ---- Device ----
    key = (C, SS)
    if key not in _NC_CACHE:
        _NC_CACHE[key] = _build_nc(C, SS)
    nc = _NC_CACHE[key]
    res = run_bass_kernel_spmd(nc, in_maps, list(range(NCORES)))
    LAST_RESULTS = res

    # ---- Combine (host; device outputs are transposed bf16 [D, Ttok]) ----
    CF, CT = (C // 512) * 512, C % 512
    out = np.empty((T, D), dtype=np.float32)
    for e in range(E):
        out[e * SS : (e + 1) * SS] = res.results[e]["ys"].astype(np.float32).T
    for e in range(E):
        yT = np.empty((D, C), dtype=np.float32)
        yT[:, :CF] = res.results[e]["yg"][:, :CF].astype(np.float32)
        if CT:
            # ygtail [128, MD*CT]: partition p, col dt*CT+j -> yT[dt*128+p, CF+j]
            tail = res.results[e]["ygtail"].astype(np.float32)
            yT[:, CF:] = tail.reshape(P, D // P, CT).transpose(1, 0, 2).reshape(D, CT)
        out[toks[e]] += wtok[e][:, None] * yT[:, : counts[e]].T
    return out.reshape(B, S, D)


# revision 43
# speedup vs baseline: 1.0115x; 1.0115x over previous
"""MoE layer (routed top-2 experts + shared SwiGLU expert) on 8 TRN2 NeuronCores.

Sharding strategy (per spec hint):
  - Routed experts: expert-parallel. Core e holds W1/W2/W3[e]; the host computes
    the router (bit-matching the reference's jax fp32 computation on CPU), gathers
    each expert's assigned tokens (top-2 of 8 per token => ~T/4 tokens per expert),
    and ships a [C, D] token block per core (C = max expert count). This is exact
    vs. the dense reference since w_full is zero for non-selected experts.
  - Shared expert: data-parallel on tokens. Core e processes tokens
    [e*T/8, (e+1)*T/8) through the full shared SwiGLU (weights replicated).
  - Combine: host scatter-add of weighted routed outputs + shared outputs.

Device kernel per core: two SwiGLU FFN passes (routed block, shared block):
    hT = (W1^T x^T) [H, Ttok]  (PSUM f32, accumulated over D/128 chunks)
    h2T = hT * silu(h3T)       (ACT sigmoid + DVE muls, cast to bf16)
    yT = W2^T h2T              [D, Ttok]  (W2 stationary, tokens streamed)
All matmuls in bf16 with fp32 PSUM accumulation; outputs shipped bf16.

Phase B keeps W2 stationary and streams tokens so the partial (C % 512) token
chunk costs cycles proportional to its width (the token-stationary form streams
all 512 W2 columns per token tile), and so the kernel's final output tile is a
tiny [128, C%512] bf16 block - the post-compute DMA drain is ~1us instead of
~8us for a [67, 512] f32 row-strided block.
"""

from contextlib import ExitStack

import numpy as np
import ml_dtypes

import concourse.bacc as bacc
import concourse.tile as tile
from concourse import mybir
from concourse.bass_utils import run_bass_kernel_spmd

# Problem constants (hardcoded per the self-contained-kernel contract)
B, S, D, H, E, TOPK = 2, 2048, 1024, 2048, 8, 2
SCALE = 1.0 / float(np.sqrt(D))
NCORES = 8
P = 128
BF16 = ml_dtypes.bfloat16

# test.py introspection: last BassKernelResults (exec_time_ns when BASS_TRACE=1)
LAST_RESULTS = None

_NC_CACHE = {}

# Sigmoid+DVE-mul beats the ACT Silu table by ~54us on HW (cold-table cost),
# and CoreSim has no Silu - so the split path is the default everywhere.
SIM_COMPAT_SILU = True

# PE p-state warm-up matmuls emitted before the first real chain. ~85ns each
# while ramping; sized so the warm-up ends about when the leading x/w3 DMAs
# land (~12us - descriptor-generation-limited). Longer warm-ups do NOT pay
# off: the cold start is delivery-limited either way, and extra warm-up is
# pure added PE busy (measured +3.4us at 72).
WARMUP_MM = 52


def _ensure_ntff_hook():
    """run_bass_kernel_spmd(trace=True) imports antenv.axon_hooks, which this
    image's antenv lacks. Install a stub (wired to the ctypes NTFF profiler if
    available) so a BASS_TRACE=1 environment doesn't crash the kernel."""
    import sys
    import types

    try:
        import antenv.axon_hooks  # noqa: F401

        return
    except ImportError:
        pass
    try:
        import antenv
    except ImportError:
        return
    mod = types.ModuleType("antenv.axon_hooks")
    holder = [None]
    mod.set_axon_ntff_profile_hook = lambda h: holder.__setitem__(0, h)
    mod.get_axon_ntff_profile_hook = lambda: holder[0]
    sys.modules["antenv.axon_hooks"] = mod
    antenv.axon_hooks = mod
    try:
        import trn_agent_boot.trn_boot as tb

        mod.set_axon_ntff_profile_hook(
            tb._ntff_profile_via_ctypes("/opt/axon/libaxon_pjrt.so")
        )
    except Exception:
        pass
    # In hook-less images the artifact share upload is likely unavailable too;
    # make the trace path's upload best-effort instead of fatal.
    try:
        import concourse.bass_utils as bu

        _orig_upload = bu.upload_artifacts

        def _safe_upload(tmpdir):
            try:
                return _orig_upload(tmpdir)
            except Exception:
                return tmpdir

        bu.upload_artifacts = _safe_upload
    except Exception:
        pass


_ensure_ntff_hook()


def _token_chunks(t, step=512):
    """[(offset, size), ...] covering range(t) in chunks of <=step."""
    out = []
    o = 0
    while o < t:
        out.append((o, min(step, t - o)))
        o += step
    return out


def _emit_ffn(tc, pools, dram, Ttok, out_qs, lead_on_scalar=False):
    """Emit one SwiGLU FFN pass: yT[D,Ttok] = W2^T (x@W1 * silu(x@W3))^T.

    dram: dict with xt [D,Ttok] bf16, w1/w3 [D,H] bf16, w2 [H,D] bf16,
          yt [D,Ttok] bf16 DRAM APs.
    out_qs: engines whose DGE queues carry the output DMAs (rotated).
    lead_on_scalar: first pass only - later passes' leads would queue behind
          the previous pass's output DMAs (gated on mid-kernel evicts).
    """
    nc = tc.nc
    KD = D // P    # contraction chunks for phase A (8)
    MH = H // P    # h tiles (16)
    KH = H // P    # contraction chunks for phase B (16)
    MD = D // P    # output D tiles for phase B (8)
    # 512-wide token chunks everywhere: one PSUM bank per tile. (A 1024-wide
    # f32 PSUM matmul passes the bass-level compile but neuronxcc rejects it.)
    chunksA = _token_chunks(Ttok, 512)
    chunksB = _token_chunks(Ttok, 512)

    xt_d = dram["xt"].rearrange("(k p) t -> k p t", p=P)     # [KD, 128, Ttok]
    w1_d = dram["w1"].rearrange("(k p) h -> k p h", p=P)     # [KD, 128, H]
    w3_d = dram["w3"].rearrange("(k p) h -> k p h", p=P)
    w2_d = dram["w2"].rearrange("(k p) d -> k p d", p=P)     # [KH, 128, D]
    yt_d = dram["yt"]                                        # [D, nfull*512]
    ytail_d = dram.get("ytail")                              # [P, MD*tail] or None

    # Resident SBUF tensors (bufs=1 pools; pass 2 reuses the same slots)
    x_sb = pools["x"].tile([P, KD, Ttok], mybir.dt.bfloat16, tag="x_sb")
    w1_sb = pools["wA"].tile([P, KD, H], mybir.dt.bfloat16, tag="w1_sb")
    w3_sb = pools["wA"].tile([P, KD, H], mybir.dt.bfloat16, tag="w3_sb")
    w2_sb = pools["wB"].tile([P, KH, D], mybir.dt.bfloat16, tag="w2_sb")
    h2t_sb = pools["h2t"].tile([P, KH, Ttok], mybir.dt.bfloat16, tag="h2t_sb")

    # DMA delivery matches phase A's consumption order: x first, then w3/w1 in
    # alternating 512-column blocks (ascending columns, all k per block) -
    # phase A's mi-th tile needs columns [mi*128,(mi+1)*128) of EVERY k, so
    # column-blocked interleave keeps the PE fed from mi=0 on instead of
    # gating mi>=4 on the whole w3 bulk. A starved PE also makes HAM downshift
    # the clock, so smooth delivery is worth ~2x here. Descriptor generation
    # costs ~0.6us per 128-line transfer serialized per sequencer (fusing k's
    # doesn't help - gen scales with lines), and the cold-start lead is
    # gen-limited: for the first pass, w3's lead block plus half of w1's ride
    # the Scalar queue in parallel with x plus the other w1 half on Sync,
    # which lands x ~12us, w3 lead ~13us, w1 lead ~15us - just in time for
    # the first ps3/ps1 chains.
    WB = 512  # weight column block (4 mi tiles)
    for k in range(KD):
        nc.sync.dma_start(out=x_sb[:, k, :], in_=xt_d[k])  # full row: widest lines
    for b0 in range(0, H, WB):
        bsl = slice(b0, b0 + WB)
        lead = b0 == 0 and lead_on_scalar
        for k in range(KD):
            q = nc.scalar if lead else nc.sync
            q.dma_start(out=w3_sb[:, k, bsl], in_=w3_d[k, :, bsl])
        for k in range(KD):
            q = (nc.sync if k < KD // 2 else nc.scalar) if lead else nc.sync
            q.dma_start(out=w1_sb[:, k, bsl], in_=w1_d[k, :, bsl])
    for k in range(KH):
        nc.sync.dma_start(out=w2_sb[:, k, :], in_=w2_d[k])

    # Phase A: h2T[H, Ttok] = (W1^T x^T) * silu(W3^T x^T), bf16.
    # k-outer per h-tile: each stationary weight chunk streams all token chunks
    # (fewer LDWEIGHTS, better hiding). All token-chunk PSUM tiles stay live.
    for mi in range(MH):
        hsl = slice(mi * P, (mi + 1) * P)
        for (o, nw) in chunksA:
            # ps3 accumulates FIRST: its sigmoid+mul evict then overlaps ps1's
            # matmuls, leaving only the final h2t mul exposed after ps1 stops.
            ps3 = pools["psA"].tile([P, 512], mybir.dt.float32, tag="ps3", bufs=2)
            ps1 = pools["psA"].tile([P, 512], mybir.dt.float32, tag="ps1")
            for k in range(KD):
                nc.tensor.matmul(
                    ps3[:, :nw],
                    lhsT=w3_sb[:, k : k + 1, hsl],
                    rhs=x_sb[:, k : k + 1, o : o + nw],
                    start=(k == 0),
                    stop=(k == KD - 1),
                )
            for k in range(KD):
                nc.tensor.matmul(
                    ps1[:, :nw],
                    lhsT=w1_sb[:, k : k + 1, hsl],
                    rhs=x_sb[:, k : k + 1, o : o + nw],
                    start=(k == 0),
                    stop=(k == KD - 1),
                )
            # silu = h3 * sigmoid(h3). The split sigmoid+mul path is both
            # CoreSim-compatible and faster on HW than ACT's Silu table.
            sil = pools["tmp"].tile([P, 512], mybir.dt.float32, tag="sil")
            if SIM_COMPAT_SILU:
                sig = pools["tmp"].tile([P, 512], mybir.dt.float32, tag="sig")
                nc.scalar.activation(
                    sig[:, :nw], ps3[:, :nw], mybir.ActivationFunctionType.Sigmoid
                )
                nc.vector.tensor_mul(sil[:, :nw], ps3[:, :nw], sig[:, :nw])
            else:
                nc.scalar.activation(
                    sil[:, :nw], ps3[:, :nw], mybir.ActivationFunctionType.Silu
                )
            nc.vector.tensor_mul(h2t_sb[:, mi, o : o + nw], ps1[:, :nw], sil[:, :nw])

    # Phase B: yT[D, Ttok] = W2^T h2T. W2 stationary, tokens streamed, so the
    # partial last chunk costs ~nw/512 of a full tile and ends the pass with a
    # small output DMA. chunk-outer / d-tile-inner; 16-matmul PSUM chains.
    qi = 0
    for (o, nw) in chunksB:
        partial = nw < 512
        if partial:
            # Partial-chunk outputs pack into one SBUF tile and ship as a
            # single [128, MD*nw] DMA: per-dt [128, nw] stores would emit
            # 128 descriptors of nw*2-byte lines each - a ~4us post-compute
            # drain for the kernel's very last tile.
            ypack = pools["tmp"].tile([P, MD * nw], mybir.dt.bfloat16, tag="ypack")
        for dt in range(MD):
            dsl = slice(dt * P, (dt + 1) * P)
            ps = pools["psB"].tile([P, 512], mybir.dt.float32, tag="psB")
            for k in range(KH):
                nc.tensor.matmul(
                    ps[:, :nw],
                    lhsT=w2_sb[:, k : k + 1, dsl],
                    rhs=h2t_sb[:, k : k + 1, o : o + nw],
                    start=(k == 0),
                    stop=(k == KH - 1),
                )
            # Output DMAs ride the GpSimd ring exclusively: descriptor gen on
            # the Scalar queue would block ACT's sigmoid stream (same
            # sequencer) right at the B->A pass transition, and Sync carries
            # the input weight stream.
            if partial:
                nc.vector.tensor_copy(
                    out=ypack[:, dt * nw : (dt + 1) * nw], in_=ps[:, :nw]
                )
            else:
                yt = pools["tmp"].tile([P, 512], mybir.dt.bfloat16, tag="yt")
                nc.vector.tensor_copy(out=yt[:, :nw], in_=ps[:, :nw])
                q = out_qs[qi % len(out_qs)]
                qi += 1
                q.dma_start(out=yt_d[dsl, o : o + nw], in_=yt[:, :nw])
        if partial:
            # Two halves so the first DMA's descriptor gen overlaps the
            # remaining pack casts instead of waiting for all MD of them.
            half = (MD // 2) * nw
            out_qs[0].dma_start(out=ytail_d[:, :half], in_=ypack[:, :half])
            out_qs[-1].dma_start(out=ytail_d[:, half:], in_=ypack[:, half:])


def _build_nc(C, SS):
    """Build the per-core Bass program: shared FFN ([SS] tokens) + routed FFN ([C])."""
    nc = bacc.Bacc("TRN2", target_bir_lowering=False, debug=False)

    bf = mybir.dt.bfloat16
    MD = D // P
    CF, CT = (C // 512) * 512, C % 512    # full-chunk cols, tail cols
    routed = {
        "xt": nc.dram_tensor("xgt", [D, C], bf, kind="ExternalInput").ap(),
        "w1": nc.dram_tensor("w1", [D, H], bf, kind="ExternalInput").ap(),
        "w3": nc.dram_tensor("w3", [D, H], bf, kind="ExternalInput").ap(),
        "w2": nc.dram_tensor("w2", [H, D], bf, kind="ExternalInput").ap(),
        "yt": nc.dram_tensor("yg", [D, max(CF, 1)], bf, kind="ExternalOutput").ap(),
    }
    if CT:
        routed["ytail"] = nc.dram_tensor(
            "ygtail", [P, MD * CT], bf, kind="ExternalOutput"
        ).ap()
    assert SS % 512 == 0, "shared block must be whole 512-token chunks"
    shared = {
        "xt": nc.dram_tensor("xst", [D, SS], bf, kind="ExternalInput").ap(),
        "w1": nc.dram_tensor("ws1", [D, H], bf, kind="ExternalInput").ap(),
        "w3": nc.dram_tensor("ws3", [D, H], bf, kind="ExternalInput").ap(),
        "w2": nc.dram_tensor("ws2", [H, D], bf, kind="ExternalInput").ap(),
        "yt": nc.dram_tensor("ys", [D, SS], bf, kind="ExternalOutput").ap(),
    }

    with tile.TileContext(nc) as tc, ExitStack() as ctx:
        pools = {
            "x": ctx.enter_context(tc.tile_pool(name="x", bufs=1)),
            "wA": ctx.enter_context(tc.tile_pool(name="wA", bufs=1)),
            "wB": ctx.enter_context(tc.tile_pool(name="wB", bufs=1)),
            "h2t": ctx.enter_context(tc.tile_pool(name="h2t", bufs=1)),
            "tmp": ctx.enter_context(tc.tile_pool(name="tmp", bufs=4)),
            "psA": ctx.enter_context(tc.tile_pool(name="psA", bufs=3, space="PSUM")),
            "psB": ctx.enter_context(tc.tile_pool(name="psB", bufs=3, space="PSUM")),
        }
        # HAM warm-up: dummy matmuls on a zeroed tile while the input DMAs
        # stream in, so the PE clock-gate is ramped when real work starts.
        warm = pools["tmp"].tile([P, P], mybir.dt.bfloat16, tag="warm")
        nc.vector.memset(warm[:], 0.0)
        wps = pools["psA"].tile([P, P], mybir.dt.float32, tag="ps1", name="wps")
        for i in range(WARMUP_MM):
            nc.tensor.matmul(wps[:], lhsT=warm[:], rhs=warm[:], start=True, stop=True)
        # All outputs ride the GpSimd ring: Scalar-ring descriptor gen blocks
        # ACT's sigmoid stream (shared sequencer), which stalls the next
        # pass's A phase; the tail is semaphore-drain dominated either way.
        _emit_ffn(tc, pools, shared, SS, [nc.gpsimd], lead_on_scalar=True)
        _emit_ffn(tc, pools, routed, C, [nc.gpsimd])

    nc.compile()
    return nc


def _route(x, Wr, rb):
    """Replicate the reference router. Returns (idx [T,2] int, w [T,2] f32).

    Uses jax on CPU with the exact expressions from the reference so the top-2
    selection bit-matches a CPU-run reference (min 2nd-vs-3rd logit gap in this
    problem is ~1e-6, so the selection must match the reference's fp32 math).
    Falls back to numpy float64 if jax-cpu is unavailable.
    """
    try:
        import jax
        import jax.numpy as jnp

        cpu = jax.devices("cpu")[0]
        with jax.default_device(cpu):
            xl = jnp.asarray(np.asarray(x))
            wr = jnp.asarray(np.asarray(Wr))
            rbj = jnp.asarray(np.asarray(rb))
            logits = jnp.einsum("bsd,de->bse", xl, wr) * SCALE
            _, idx = jax.lax.top_k(logits + rbj, TOPK)
            gathered = jnp.take_along_axis(logits, idx, axis=-1)
            w = jax.nn.softmax(gathered, axis=-1)
        idx = np.asarray(idx).reshape(-1, TOPK)
        w = np.asarray(w, dtype=np.float32).reshape(-1, TOPK)
        return idx, w
    except Exception:
        xf = np.asarray(x, np.float64).reshape(-1, D)
        logits = (xf @ np.asarray(Wr, np.float64)) * SCALE
        biased = logits + np.asarray(rb, np.float64)
        idx = np.argsort(-biased, axis=-1)[:, :TOPK]
        g = np.take_along_axis(logits, idx, axis=-1)
        g = g - g.max(axis=-1, keepdims=True)
        wexp = np.exp(g)
        w = (wexp / wexp.sum(axis=-1, keepdims=True)).astype(np.float32)
        return idx, w


def kernel(x, Wr, rb, W1, W2, W3, Ws1, Ws2, Ws3):
    global LAST_RESULTS
    x = np.asarray(x, np.float32)
    Wr = np.asarray(Wr, np.float32)
    rb = np.asarray(rb, np.float32)
    W1 = np.asarray(W1, np.float32)
    W2 = np.asarray(W2, np.float32)
    W3 = np.asarray(W3, np.float32)
    Ws1 = np.asarray(Ws1, np.float32)
    Ws2 = np.asarray(Ws2, np.float32)
    Ws3 = np.asarray(Ws3, np.float32)

    T = B * S
    xf = x.reshape(T, D)

    # ---- Router (host, exact) ----
    idx, w = _route(x, Wr, rb)

    # ---- Shard ----
    toks = [np.nonzero((idx == e).any(axis=1))[0] for e in range(E)]
    wtok = [
        w[toks[e], :][idx[toks[e], :] == e].astype(np.float32) for e in range(E)
    ]
    counts = [len(t) for t in toks]
    C = max(256, max(counts))  # exact max count; matmul free dims need no alignment
    SS = T // NCORES

    xf_bf = xf.astype(BF16)
    in_maps = []
    for e in range(E):
        xg = np.zeros((C, D), dtype=BF16)
        xg[: counts[e]] = xf_bf[toks[e]]
        in_maps.append(
            {
                "xgt": np.ascontiguousarray(xg.T),
                "w1": np.ascontiguousarray(W1[e].astype(BF16)),
                "w3": np.ascontiguousarray(W3[e].astype(BF16)),
                "w2": np.ascontiguousarray(W2[e].astype(BF16)),
                "xst": np.ascontiguousarray(xf_bf[e * SS : (e + 1) * SS].T),
                "ws1": np.ascontiguousarray(Ws1.astype(BF16)),
                "ws3": np.ascontiguousarray(Ws3.astype(BF16)),
                "ws2": np.ascontiguousarray(Ws2.astype(BF16)),
            }
        )

    # ---- Device ----
    key = (C, SS)
    if key not in _NC_CACHE:
        _NC_CACHE[key] = _build_nc(C, SS)
    nc = _NC_CACHE[key]
    res = run_bass_kernel_spmd(nc, in_maps, list(range(NCORES)))
    LAST_RESULTS = res

    # ---- Combine (host; device outputs are transposed bf16 [D, Ttok]) ----
    CF, CT = (C // 512) * 512, C % 512
    out = np.empty((T, D), dtype=np.float32)
    for e in range(E):
        out[e * SS : (e + 1) * SS] = res.results[e]["ys"].astype(np.float32).T
    for e in range(E):
        yT = np.empty((D, C), dtype=np.float32)
        yT[:, :CF] = res.results[e]["yg"][:, :CF].astype(np.float32)
        if CT:
            # ygtail [128, MD*CT]: partition p, col dt*CT+j -> yT[dt*128+p, CF+j]
            tail = res.results[e]["ygtail"].astype(np.float32)
            yT[:, CF:] = tail.reshape(P, D // P, CT).transpose(1, 0, 2).reshape(D, CT)
        out[toks[e]] += wtok[e][:, None] * yT[:, : counts[e]].T
    return out.reshape(B, S, D)


# revision 46
# speedup vs baseline: 1.0212x; 1.0095x over previous
"""MoE layer (routed top-2 experts + shared SwiGLU expert) on 8 TRN2 NeuronCores.

Sharding strategy (per spec hint):
  - Routed experts: expert-parallel. Core e holds W1/W2/W3[e]; the host computes
    the router (bit-matching the reference's jax fp32 computation on CPU), gathers
    each expert's assigned tokens (top-2 of 8 per token => ~T/4 tokens per expert),
    and ships a [C, D] token block per core (C = max expert count). This is exact
    vs. the dense reference since w_full is zero for non-selected experts.
  - Shared expert: data-parallel on tokens. Core e processes tokens
    [e*T/8, (e+1)*T/8) through the full shared SwiGLU (weights replicated).
  - Combine: host scatter-add of weighted routed outputs + shared outputs.

Device kernel per core: two SwiGLU FFN passes (routed block, shared block):
    hT = (W1^T x^T) [H, Ttok]  (PSUM f32, accumulated over D/128 chunks)
    h2T = hT * silu(h3T)       (ACT sigmoid + DVE muls, cast to bf16)
    yT = W2^T h2T              [D, Ttok]  (W2 stationary, tokens streamed)
All matmuls in bf16 with fp32 PSUM accumulation; outputs shipped bf16.

Phase B keeps W2 stationary and streams tokens so the partial (C % 512) token
chunk costs cycles proportional to its width (the token-stationary form streams
all 512 W2 columns per token tile), and so the kernel's final output tile is a
tiny [128, C%512] bf16 block - the post-compute DMA drain is ~1us instead of
~8us for a [67, 512] f32 row-strided block.
"""

from contextlib import ExitStack

import numpy as np
import ml_dtypes

import concourse.bacc as bacc
import concourse.tile as tile
from concourse import mybir
from concourse.bass_utils import run_bass_kernel_spmd

# Problem constants (hardcoded per the self-contained-kernel contract)
B, S, D, H, E, TOPK = 2, 2048, 1024, 2048, 8, 2
SCALE = 1.0 / float(np.sqrt(D))
NCORES = 8
P = 128
BF16 = ml_dtypes.bfloat16

# test.py introspection: last BassKernelResults (exec_time_ns when BASS_TRACE=1)
LAST_RESULTS = None

_NC_CACHE = {}

# Sigmoid+DVE-mul beats the ACT Silu table by ~54us on HW (cold-table cost),
# and CoreSim has no Silu - so the split path is the default everywhere.
SIM_COMPAT_SILU = True

# PE p-state warm-up matmuls emitted before the first real chain. ~85ns each
# while ramping; sized so the warm-up ends about when the leading x/w3 DMAs
# land (~12us - descriptor-generation-limited). Longer warm-ups do NOT pay
# off: the cold start is delivery-limited either way, and extra warm-up is
# pure added PE busy (measured +3.4us at 72).
WARMUP_MM = 52


def _ensure_ntff_hook():
    """run_bass_kernel_spmd(trace=True) imports antenv.axon_hooks, which this
    image's antenv lacks. Install a stub (wired to the ctypes NTFF profiler if
    available) so a BASS_TRACE=1 environment doesn't crash the kernel."""
    import sys
    import types

    try:
        import antenv.axon_hooks  # noqa: F401

        return
    except ImportError:
        pass
    try:
        import antenv
    except ImportError:
        return
    mod = types.ModuleType("antenv.axon_hooks")
    holder = [None]
    mod.set_axon_ntff_profile_hook = lambda h: holder.__setitem__(0, h)
    mod.get_axon_ntff_profile_hook = lambda: holder[0]
    sys.modules["antenv.axon_hooks"] = mod
    antenv.axon_hooks = mod
    try:
        import trn_agent_boot.trn_boot as tb

        mod.set_axon_ntff_profile_hook(
            tb._ntff_profile_via_ctypes("/opt/axon/libaxon_pjrt.so")
        )
    except Exception:
        pass
    # In hook-less images the artifact share upload is likely unavailable too;
    # make the trace path's upload best-effort instead of fatal.
    try:
        import concourse.bass_utils as bu

        _orig_upload = bu.upload_artifacts

        def _safe_upload(tmpdir):
            try:
                return _orig_upload(tmpdir)
            except Exception:
                return tmpdir

        bu.upload_artifacts = _safe_upload
    except Exception:
        pass


_ensure_ntff_hook()


def _token_chunks(t, step=512):
    """[(offset, size), ...] covering range(t) in chunks of <=step."""
    out = []
    o = 0
    while o < t:
        out.append((o, min(step, t - o)))
        o += step
    return out


def _emit_ffn(tc, pools, dram, Ttok, out_qs, lead_on_scalar=False):
    """Emit one SwiGLU FFN pass: yT[D,Ttok] = W2^T (x@W1 * silu(x@W3))^T.

    dram: dict with xt [D,Ttok] bf16, w1/w3 [D,H] bf16, w2 [H,D] bf16,
          yt [D,Ttok] bf16 DRAM APs.
    out_qs: engines whose DGE queues carry the output DMAs (rotated).
    lead_on_scalar: first pass only - later passes' leads would queue behind
          the previous pass's output DMAs (gated on mid-kernel evicts).
    """
    nc = tc.nc
    KD = D // P    # contraction chunks for phase A (8)
    MH = H // P    # h tiles (16)
    KH = H // P    # contraction chunks for phase B (16)
    MD = D // P    # output D tiles for phase B (8)
    # 512-wide token chunks everywhere: one PSUM bank per tile. (A 1024-wide
    # f32 PSUM matmul passes the bass-level compile but neuronxcc rejects it.)
    chunksA = _token_chunks(Ttok, 512)
    chunksB = _token_chunks(Ttok, 512)

    xt_d = dram["xt"].rearrange("(k p) t -> k p t", p=P)     # [KD, 128, Ttok]
    w1_d = dram["w1"].rearrange("(k p) h -> k p h", p=P)     # [KD, 128, H]
    w3_d = dram["w3"].rearrange("(k p) h -> k p h", p=P)
    w2_d = dram["w2"].rearrange("(k p) d -> k p d", p=P)     # [KH, 128, D]
    yt_d = dram["yt"]                                        # [D, nfull*512]
    ytail_d = dram.get("ytail")                              # [P, MD*tail] or None

    # Resident SBUF tensors (bufs=1 pools; pass 2 reuses the same slots)
    x_sb = pools["x"].tile([P, KD, Ttok], mybir.dt.bfloat16, tag="x_sb")
    w1_sb = pools["wA"].tile([P, KD, H], mybir.dt.bfloat16, tag="w1_sb")
    w3_sb = pools["wA"].tile([P, KD, H], mybir.dt.bfloat16, tag="w3_sb")
    w2_sb = pools["wB"].tile([P, KH, D], mybir.dt.bfloat16, tag="w2_sb")
    h2t_sb = pools["h2t"].tile([P, KH, Ttok], mybir.dt.bfloat16, tag="h2t_sb")

    # DMA delivery matches phase A's consumption order: x first, then w3/w1 in
    # alternating 512-column blocks (ascending columns, all k per block) -
    # phase A's mi-th tile needs columns [mi*128,(mi+1)*128) of EVERY k, so
    # column-blocked interleave keeps the PE fed from mi=0 on instead of
    # gating mi>=4 on the whole w3 bulk. A starved PE also makes HAM downshift
    # the clock, so smooth delivery is worth ~2x here. Descriptor generation
    # costs ~0.6us per 128-line transfer serialized per sequencer (fusing k's
    # doesn't help - gen scales with lines), and the cold-start lead is
    # gen-limited: for the first pass, w3's lead block plus half of w1's ride
    # the Scalar queue in parallel with x plus the other w1 half on Sync,
    # which lands x ~12us, w3 lead ~13us, w1 lead ~15us - just in time for
    # the first ps3/ps1 chains.
    WB = 512  # weight column block (4 mi tiles)
    for k in range(KD):
        nc.sync.dma_start(out=x_sb[:, k, :], in_=xt_d[k])  # full row: widest lines
    for b0 in range(0, H, WB):
        bsl = slice(b0, b0 + WB)
        lead = b0 == 0 and lead_on_scalar
        for k in range(KD):
            q = nc.scalar if lead else nc.sync
            q.dma_start(out=w3_sb[:, k, bsl], in_=w3_d[k, :, bsl])
        for k in range(KD):
            q = (nc.sync if k < KD // 2 else nc.scalar) if lead else nc.sync
            q.dma_start(out=w1_sb[:, k, bsl], in_=w1_d[k, :, bsl])
    for k in range(KH):
        nc.sync.dma_start(out=w2_sb[:, k, :], in_=w2_d[k])

    # Phase A: h2T[H, Ttok] = (W1^T x^T) * silu(W3^T x^T), bf16.
    # On the first (cold-start) pass the ps3 chains run SK units ahead of the
    # ps1 chains: the lead is descriptor-gen-limited and w1's block lands
    # ~2-3us after w3's, so the first SK chains of real work need only x+w3.
    # Later passes have their weights long resident (SK=0: plain ps3/ps1
    # pairs). The sil tiles buffer the skew until the ps1 partner lands.
    SK = 3 if lead_on_scalar else 0
    units = [(mi, o, nw) for mi in range(MH) for (o, nw) in chunksA]
    sils = {}

    def emit_ps3(u):
        mi, o, nw = units[u]
        hsl = slice(mi * P, (mi + 1) * P)
        ps3 = pools["psA"].tile([P, 512], mybir.dt.float32, tag="ps3", bufs=3)
        for k in range(KD):
            nc.tensor.matmul(
                ps3[:, :nw],
                lhsT=w3_sb[:, k : k + 1, hsl],
                rhs=x_sb[:, k : k + 1, o : o + nw],
                start=(k == 0),
                stop=(k == KD - 1),
            )
        # silu = h3 * sigmoid(h3). The split sigmoid+mul path is both
        # CoreSim-compatible and faster on HW than ACT's Silu table.
        sil = pools["tmp"].tile([P, 512], mybir.dt.float32, tag="sil", bufs=5)
        if SIM_COMPAT_SILU:
            sig = pools["tmp"].tile([P, 512], mybir.dt.float32, tag="sig")
            nc.scalar.activation(
                sig[:, :nw], ps3[:, :nw], mybir.ActivationFunctionType.Sigmoid
            )
            nc.vector.tensor_mul(sil[:, :nw], ps3[:, :nw], sig[:, :nw])
        else:
            nc.scalar.activation(
                sil[:, :nw], ps3[:, :nw], mybir.ActivationFunctionType.Silu
            )
        sils[u] = sil

    def emit_ps1(u):
        mi, o, nw = units[u]
        hsl = slice(mi * P, (mi + 1) * P)
        ps1 = pools["psA"].tile([P, 512], mybir.dt.float32, tag="ps1", bufs=2)
        for k in range(KD):
            nc.tensor.matmul(
                ps1[:, :nw],
                lhsT=w1_sb[:, k : k + 1, hsl],
                rhs=x_sb[:, k : k + 1, o : o + nw],
                start=(k == 0),
                stop=(k == KD - 1),
            )
        sil = sils.pop(u)
        nc.vector.tensor_mul(h2t_sb[:, mi, o : o + nw], ps1[:, :nw], sil[:, :nw])

    for u in range(len(units) + SK):
        if u < len(units):
            emit_ps3(u)
        if u >= SK:
            emit_ps1(u - SK)

    # Phase B: yT[D, Ttok] = W2^T h2T. W2 stationary, tokens streamed, so the
    # partial last chunk costs ~nw/512 of a full tile and ends the pass with a
    # small output DMA. chunk-outer / d-tile-inner; 16-matmul PSUM chains.
    qi = 0
    for (o, nw) in chunksB:
        partial = nw < 512
        if partial:
            # Partial-chunk outputs pack into one SBUF tile and ship as a
            # single [128, MD*nw] DMA: per-dt [128, nw] stores would emit
            # 128 descriptors of nw*2-byte lines each - a ~4us post-compute
            # drain for the kernel's very last tile.
            ypack = pools["tmp"].tile([P, MD * nw], mybir.dt.bfloat16, tag="ypack")
        for dt in range(MD):
            dsl = slice(dt * P, (dt + 1) * P)
            ps = pools["psB"].tile([P, 512], mybir.dt.float32, tag="psB")
            for k in range(KH):
                nc.tensor.matmul(
                    ps[:, :nw],
                    lhsT=w2_sb[:, k : k + 1, dsl],
                    rhs=h2t_sb[:, k : k + 1, o : o + nw],
                    start=(k == 0),
                    stop=(k == KH - 1),
                )
            # Output DMAs ride the GpSimd ring exclusively: descriptor gen on
            # the Scalar queue would block ACT's sigmoid stream (same
            # sequencer) right at the B->A pass transition, and Sync carries
            # the input weight stream.
            if partial:
                nc.vector.tensor_copy(
                    out=ypack[:, dt * nw : (dt + 1) * nw], in_=ps[:, :nw]
                )
            else:
                yt = pools["tmp"].tile([P, 512], mybir.dt.bfloat16, tag="yt")
                nc.vector.tensor_copy(out=yt[:, :nw], in_=ps[:, :nw])
                q = out_qs[qi % len(out_qs)]
                qi += 1
                q.dma_start(out=yt_d[dsl, o : o + nw], in_=yt[:, :nw])
        if partial:
            # Two halves so the first DMA's descriptor gen overlaps the
            # remaining pack casts instead of waiting for all MD of them.
            half = (MD // 2) * nw
            out_qs[0].dma_start(out=ytail_d[:, :half], in_=ypack[:, :half])
            out_qs[-1].dma_start(out=ytail_d[:, half:], in_=ypack[:, half:])


def _build_nc(C, SS):
    """Build the per-core Bass program: shared FFN ([SS] tokens) + routed FFN ([C])."""
    nc = bacc.Bacc("TRN2", target_bir_lowering=False, debug=False)

    bf = mybir.dt.bfloat16
    MD = D // P
    CF, CT = (C // 512) * 512, C % 512    # full-chunk cols, tail cols
    routed = {
        "xt": nc.dram_tensor("xgt", [D, C], bf, kind="ExternalInput").ap(),
        "w1": nc.dram_tensor("w1", [D, H], bf, kind="ExternalInput").ap(),
        "w3": nc.dram_tensor("w3", [D, H], bf, kind="ExternalInput").ap(),
        "w2": nc.dram_tensor("w2", [H, D], bf, kind="ExternalInput").ap(),
        "yt": nc.dram_tensor("yg", [D, max(CF, 1)], bf, kind="ExternalOutput").ap(),
    }
    if CT:
        routed["ytail"] = nc.dram_tensor(
            "ygtail", [P, MD * CT], bf, kind="ExternalOutput"
        ).ap()
    assert SS % 512 == 0, "shared block must be whole 512-token chunks"
    shared = {
        "xt": nc.dram_tensor("xst", [D, SS], bf, kind="ExternalInput").ap(),
        "w1": nc.dram_tensor("ws1", [D, H], bf, kind="ExternalInput").ap(),
        "w3": nc.dram_tensor("ws3", [D, H], bf, kind="ExternalInput").ap(),
        "w2": nc.dram_tensor("ws2", [H, D], bf, kind="ExternalInput").ap(),
        "yt": nc.dram_tensor("ys", [D, SS], bf, kind="ExternalOutput").ap(),
    }

    with tile.TileContext(nc) as tc, ExitStack() as ctx:
        pools = {
            "x": ctx.enter_context(tc.tile_pool(name="x", bufs=1)),
            "wA": ctx.enter_context(tc.tile_pool(name="wA", bufs=1)),
            "wB": ctx.enter_context(tc.tile_pool(name="wB", bufs=1)),
            "h2t": ctx.enter_context(tc.tile_pool(name="h2t", bufs=1)),
            "tmp": ctx.enter_context(tc.tile_pool(name="tmp", bufs=4)),
            "psA": ctx.enter_context(tc.tile_pool(name="psA", bufs=3, space="PSUM")),
            "psB": ctx.enter_context(tc.tile_pool(name="psB", bufs=3, space="PSUM")),
        }
        # HAM warm-up: dummy matmuls on a zeroed tile while the input DMAs
        # stream in, so the PE clock-gate is ramped when real work starts.
        warm = pools["tmp"].tile([P, P], mybir.dt.bfloat16, tag="warm")
        nc.vector.memset(warm[:], 0.0)
        wps = pools["psA"].tile([P, P], mybir.dt.float32, tag="ps1", bufs=2, name="wps")
        for i in range(WARMUP_MM):
            nc.tensor.matmul(wps[:], lhsT=warm[:], rhs=warm[:], start=True, stop=True)
        # All outputs ride the GpSimd ring: Scalar-ring descriptor gen blocks
        # ACT's sigmoid stream (shared sequencer), which stalls the next
        # pass's A phase; the tail is semaphore-drain dominated either way.
        _emit_ffn(tc, pools, shared, SS, [nc.gpsimd], lead_on_scalar=True)
        _emit_ffn(tc, pools, routed, C, [nc.gpsimd])

    nc.compile()
    return nc


def _route(x, Wr, rb):
    """Replicate the reference router. Returns (idx [T,2] int, w [T,2] f32).

    Uses jax on CPU with the exact expressions from the reference so the top-2
    selection bit-matches a CPU-run reference (min 2nd-vs-3rd logit gap in this
    problem is ~1e-6, so the selection must match the reference's fp32 math).
    Falls back to numpy float64 if jax-cpu is unavailable.
    """
    try:
        import jax
        import jax.numpy as jnp

        cpu = jax.devices("cpu")[0]
        with jax.default_device(cpu):
            xl = jnp.asarray(np.asarray(x))
            wr = jnp.asarray(np.asarray(Wr))
            rbj = jnp.asarray(np.asarray(rb))
            logits = jnp.einsum("bsd,de->bse", xl, wr) * SCALE
            _, idx = jax.lax.top_k(logits + rbj, TOPK)
            gathered = jnp.take_along_axis(logits, idx, axis=-1)
            w = jax.nn.softmax(gathered, axis=-1)
        idx = np.asarray(idx).reshape(-1, TOPK)
        w = np.asarray(w, dtype=np.float32).reshape(-1, TOPK)
        return idx, w
    except Exception:
        xf = np.asarray(x, np.float64).reshape(-1, D)
        logits = (xf @ np.asarray(Wr, np.float64)) * SCALE
        biased = logits + np.asarray(rb, np.float64)
        idx = np.argsort(-biased, axis=-1)[:, :TOPK]
        g = np.take_along_axis(logits, idx, axis=-1)
        g = g - g.max(axis=-1, keepdims=True)
        wexp = np.exp(g)
        w = (wexp / wexp.sum(axis=-1, keepdims=True)).astype(np.float32)
        return idx, w


def kernel(x, Wr, rb, W1, W2, W3, Ws1, Ws2, Ws3):
    global LAST_RESULTS
    x = np.asarray(x, np.float32)
    Wr = np.asarray(Wr, np.float32)
    rb = np.asarray(rb, np.float32)
    W1 = np.asarray(W1, np.float32)
    W2 = np.asarray(W2, np.float32)
    W3 = np.asarray(W3, np.float32)
    Ws1 = np.asarray(Ws1, np.float32)
    Ws2 = np.asarray(Ws2, np.float32)
    Ws3 = np.asarray(Ws3, np.float32)

    T = B * S
    xf = x.reshape(T, D)

    # ---- Router (host, exact) ----
    idx, w = _route(x, Wr, rb)

    # ---- Shard ----
    toks = [np.nonzero((idx == e).any(axis=1))[0] for e in range(E)]
    wtok = [
        w[toks[e], :][idx[toks[e], :] == e].astype(np.float32) for e in range(E)
    ]
    counts = [len(t) for t in toks]
    C = max(256, max(counts))  # exact max count; matmul free dims need no alignment
    SS = T // NCORES

    xf_bf = xf.astype(BF16)
    in_maps = []
    for e in range(E):
        xg = np.zeros((C, D), dtype=BF16)
        xg[: counts[e]] = xf_bf[toks[e]]
        in_maps.append(
            {
                "xgt": np.ascontiguousarray(xg.T),
                "w1": np.ascontiguousarray(W1[e].astype(BF16)),
                "w3": np.ascontiguousarray(W3[e].astype(BF16)),
                "w2": np.ascontiguousarray(W2[e].astype(BF16)),
                "xst": np.ascontiguousarray(xf_bf[e * SS : (e + 1) * SS].T),
                "ws1": np.ascontiguousarray(Ws1.astype(BF16)),
                "ws3": np.ascontiguousarray(Ws3.astype(BF16)),
                "ws2": np.ascontiguousarray(Ws2.astype(BF16)),
            }
        )

    # ---- Device ----
    key = (C, SS)
    if key not in _NC_CACHE:
        _NC_CACHE[key] = _build_nc(C, SS)
    nc = _NC_CACHE[key]
    res = run_bass_kernel_spmd(nc, in_maps, list(range(NCORES)))
    LAST_RESULTS = res

    # ---- Combine (host; device outputs are transposed bf16 [D, Ttok]) ----
    CF, CT = (C // 512) * 512, C % 512
    out = np.empty((T, D), dtype=np.float32)
    for e in range(E):
        out[e * SS : (e + 1) * SS] = res.results[e]["ys"].astype(np.float32).T
    for e in range(E):
        yT = np.empty((D, C), dtype=np.float32)
        yT[:, :CF] = res.results[e]["yg"][:, :CF].astype(np.float32)
        if CT:
            # ygtail [128, MD*CT]: partition p, col dt*CT+j -> yT[dt*128+p, CF+j]
            tail = res.results[e]["ygtail"].astype(np.float32)
            yT[:, CF:] = tail.reshape(P, D // P, CT).transpose(1, 0, 2).reshape(D, CT)
        out[toks[e]] += wtok[e][:, None] * yT[:, : counts[e]].T
    return out.reshape(B, S, D)
